# revision 1
# baseline (speedup 1.0000x reference)
"""Trainium2 Bass kernel for nn_PizzaBurningEffect.

Reference computation (per batch b):
    ew[h,w]   : fixed edge-weight grid (input-independent)
    spots     = max_s exp(-((x_w-sx)^2+(y_h-sy)^2)/(2 r_s^2)) * sint_s
    bm        = clip(max(ew, spots) * burn_b, 0, 1)
    out[c]    = clip(img[c]*(1-bm) + img[c]*dark_c*bm, 0, 1)
            = img[c] * (1 + fsc_{b,c} * max(ew, spots)),  fsc = -burn_b*(1-dark_c)
(The clips are no-ops: every operand is in [0,1) and bm <= 0.8.)

The Gaussian is separable: exp(-(dx^2+dy^2)/2r^2) = gx(w) * gy(h), so the
device only does an outer-product max over 8 spots plus the blend; the tiny
1-D tables gx/gy (B*S*512 floats) are exp'd on the host.

Sharding: pure data parallel, 4 batches per core on 8 cores.
"""

import numpy as np

import concourse.bacc as bacc
import concourse.bass as bass
from concourse import mybir
from concourse.tile import TileContext
from concourse.bass_utils import run_bass_kernel_spmd

B, C, H, W, S = 32, 3, 512, 512, 8
NCORES = 8
BL = B // NCORES          # batches per core
P = 128                   # partitions
K = H // P                # row chunks per image
DT = mybir.dt.float32

BURN_MIN, BURN_MAX = 0.2, 0.8
DARK = np.array([0.7, 0.4, 0.3], dtype=np.float64)

# NOTE: this walrus build rejects tensor_tensor/tensor_scalar on Pool
# (NCC_IXCG966), so all 2-src elementwise work lives on DVE; ACT handles the
# 1-src scale+bias ops.
#
# The spot-merge chain runs in fp16: 16-bit operands give DVE its 2x packing
# mode, halving the dominant cost. Mask values live in [0,1] so fp16 loses
# ~5e-4 relative on the mask -> ~3e-4 absolute on the output (measured).
# img, the blend multiply, and the output stay fp32.
DTH = mybir.dt.float16


def _build_program():
    nc = bacc.Bacc("TRN2", target_bir_lowering=False, debug=False,
                   num_devices=NCORES)

    img = nc.dram_tensor("img", [BL, C, H, W], DT, kind="ExternalInput")
    gx = nc.dram_tensor("gx", [BL, S * W], DTH, kind="ExternalInput")
    gy = nc.dram_tensor("gy", [P, BL, K, S], DT, kind="ExternalInput")
    ew = nc.dram_tensor("ew", [P, K, W], DTH, kind="ExternalInput")
    fsc = nc.dram_tensor("fsc", [P, BL, C], DT, kind="ExternalInput")
    out = nc.dram_tensor("out", [BL, C, H, W], DT, kind="ExternalOutput")

    img_r = img.rearrange("b c (k p) w -> b k p c w", p=P)
    out_r = out.rearrange("b c (k p) w -> b k p c w", p=P)

    mx = mybir.AluOpType.max
    mult = mybir.AluOpType.mult

    # spots 0..DVE_S-1 run as fused mult+max (scalar_tensor_tensor, 1x rate);
    # spots DVE_S..7 are multiplied on the scalar engine (ACT has spare
    # capacity) and merged with fp16 tensor_tensor maxes (2x packed rate).
    DVE_S = 4

    with TileContext(nc) as tc:
        with (
            tc.tile_pool(name="singles", bufs=1) as singles,
            tc.tile_pool(name="gxp", bufs=2) as gxp,
            tc.tile_pool(name="imgp", bufs=4) as imgp,
            tc.tile_pool(name="outp", bufs=4) as outp,
            tc.tile_pool(name="maskp", bufs=4) as maskp,
            tc.tile_pool(name="tmpp", bufs=4) as tmpp,
            tc.tile_pool(name="fp", bufs=4) as fp,
        ):
            # DMA issue order = chunk-0 critical path first: gy scalars,
            # ew chunk-0 slice, then (inside the b=0 iteration) both gxb
            # halves — everything else after.
            gy_t = singles.tile([P, BL, K, S], DT)
            nc.sync.dma_start(out=gy_t[:], in_=gy[:])
            ew_t = singles.tile([P, K, W], DTH)
            nc.sync.dma_start(out=ew_t[:, 0, :], in_=ew[:, 0, :])
            fsc_t = singles.tile([P, BL, C], DT)
            nc.sync.dma_start(out=fsc_t[:], in_=fsc[:])
            # Touch the scalar engine once right away so its function-table
            # load (~1.3us) overlaps the initial DMAs instead of stalling the
            # first real ACTIVATE.
            warm = singles.tile([P, 1], DT)
            nc.scalar.mul(warm[:], fsc_t[:, 0, 0:1], 1.0)

            for b in range(BL):
                gxb = gxp.tile([P, S, W], DTH)
                hw = S * W // 2
                gxf = gxb[:].rearrange("p s w -> p (s w)")
                nc.sync.dma_start(
                    out=gxf[:, 0:hw],
                    in_=gx[b][None, 0:hw].to_broadcast([P, hw]))
                nc.sync.dma_start(
                    out=gxf[:, hw:2 * hw],
                    in_=gx[b][None, hw:2 * hw].to_broadcast([P, hw]))
                if b == 0:
                    # remaining ew slices, off the chunk-0 critical path
                    for k in range(1, K):
                        nc.sync.dma_start(out=ew_t[:, k, :], in_=ew[:, k, :])
                for k in range(K):
                    img_t = imgp.tile([P, C, W], DT)
                    nc.sync.dma_start(out=img_t[:], in_=img_r[b, k])

                    # ACT: tmp_j = gx_s * gy_s for the offloaded spots
                    tmp = tmpp.tile([P, S - DVE_S, W], DTH)
                    for j, s in enumerate(range(DVE_S, S)):
                        nc.scalar.activation(
                            out=tmp[:, j, :], in_=gxb[:, s, :],
                            func=mybir.ActivationFunctionType.Copy,
                            bias=0.0, scale=gy_t[:, b, k, s:s + 1])

                    acc = maskp.tile([P, W], DTH)
                    # acc = max(gx_0 * gy_0, ew)  (ew folded into first stt)
                    nc.vector.scalar_tensor_tensor(
                        out=acc[:], in0=gxb[:, 0, :],
                        scalar=gy_t[:, b, k, 0:1], in1=ew_t[:, k, :],
                        op0=mult, op1=mx)
                    for s in range(1, DVE_S):
                        nc.vector.scalar_tensor_tensor(
                            out=acc[:], in0=gxb[:, s, :],
                            scalar=gy_t[:, b, k, s:s + 1], in1=acc[:],
                            op0=mult, op1=mx)
                    # merge ACT products: balanced fp16 tt-max tree
                    u = tmpp.tile([P, 2, W], DTH, tag="u")
                    nc.vector.tensor_tensor(
                        out=u[:, 0, :], in0=tmp[:, 0, :], in1=tmp[:, 1, :],
                        op=mx)
                    nc.vector.tensor_tensor(
                        out=u[:, 1, :], in0=tmp[:, 2, :], in1=tmp[:, 3, :],
                        op=mx)
                    nc.vector.tensor_tensor(
                        out=acc[:], in0=acc[:], in1=u[:, 0, :], op=mx)
                    nc.vector.tensor_tensor(
                        out=acc[:], in0=acc[:], in1=u[:, 1, :], op=mx)

                    # f_c = 1 + fsc_{b,c} * acc   (scalar engine)
                    f_t = fp.tile([P, C, W], DT)
                    for c in range(C):
                        nc.scalar.activation(
                            out=f_t[:, c, :], in_=acc[:],
                            func=mybir.ActivationFunctionType.Identity,
                            bias=1.0, scale=fsc_t[:, b, c:c + 1])

                    # out_c = img_c * f_c  (one wide DVE op over all channels)
                    out_t = outp.tile([P, C, W], DT)
                    nc.vector.tensor_tensor(
                        out=out_t[:], in0=img_t[:], in1=f_t[:], op=mult)

                    nc.sync.dma_start(out=out_r[b, k], in_=out_t[:])

    nc.compile()
    return nc


_NC = None


def _get_nc():
    global _NC
    if _NC is None:
        _NC = _build_program()
    return _NC


def _host_tables(u_xy, u_radius, u_intensity, u_burn):
    """Tiny 1-D exp tables, computed in float64 then cast to float32."""
    u_xy = np.asarray(u_xy, np.float64)
    u_radius = np.asarray(u_radius, np.float64)
    u_intensity = np.asarray(u_intensity, np.float64)
    u_burn = np.asarray(u_burn, np.float64)

    y = np.linspace(-1.0, 1.0, H)          # [H]
    x = np.linspace(-1.0, 1.0, W)          # [W]

    spot_xy = 2.0 * u_xy - 1.0             # [B,S,2]
    sx = spot_xy[..., 0]                   # [B,S]
    sy = spot_xy[..., 1]
    radius = 0.05 + 0.15 * u_radius        # [B,S]
    sint = 0.5 + 0.5 * u_intensity         # [B,S]
    inv2r2 = 1.0 / (2.0 * radius ** 2)     # [B,S]

    # gy[b,s,h], gx[b,s,w] (sint folded into gx)
    gy = np.exp(-((y[None, None, :] - sy[..., None]) ** 2) * inv2r2[..., None])
    gxv = np.exp(-((x[None, None, :] - sx[..., None]) ** 2) * inv2r2[..., None])
    gxv = gxv * sint[..., None]

    burn = BURN_MIN + (BURN_MAX - BURN_MIN) * u_burn   # [B]
    fsc = -(burn[:, None] * (1.0 - DARK)[None, :])     # [B,C]

    # layouts the device program expects
    # gy_lay[p, b, k, s] = gy[b, s, k*P+p]
    gy_lay = gy.reshape(B, S, K, P).transpose(3, 0, 2, 1).astype(np.float32)
    gx_lay = gxv.reshape(B, S * W).astype(np.float16)
    fsc_lay = np.broadcast_to(fsc.astype(np.float32), (P, B, C))
    return np.ascontiguousarray(gy_lay), np.ascontiguousarray(gx_lay), \
        np.ascontiguousarray(fsc_lay)


def _edge_weight():
    y = np.linspace(-1.0, 1.0, H)
    x = np.linspace(-1.0, 1.0, W)
    yc, xc = np.meshgrid(y, x, indexing="ij")
    dist = np.sqrt(xc ** 2 + yc ** 2)
    ew = np.exp(2.0 * (dist - 0.7))
    ew = (ew - ew.min()) / (ew.max() - ew.min() + 1e-6)
    # ew_lay[p, k, w] = ew[k*P+p, w]
    return np.ascontiguousarray(
        ew.reshape(K, P, W).transpose(1, 0, 2).astype(np.float16))


_EW = None


def kernel(img, u_xy, u_radius, u_intensity, u_burn, _run_kwargs=None):
    global _EW
    img = np.ascontiguousarray(np.asarray(img, np.float32))
    gy_lay, gx_lay, fsc_lay = _host_tables(u_xy, u_radius, u_intensity, u_burn)
    if _EW is None:
        _EW = _edge_weight()

    nc = _get_nc()
    core_ids = list(range(NCORES))
    in_maps = []
    for i in core_ids:
        lo, hi = i * BL, (i + 1) * BL
        in_maps.append({
            "img": img[lo:hi],
            "gx": gx_lay[lo:hi],
            "gy": np.ascontiguousarray(gy_lay[:, lo:hi]),
            "ew": _EW,
            "fsc": np.ascontiguousarray(fsc_lay[:, lo:hi]),
        })
    res = run_bass_kernel_spmd(nc, in_maps, core_ids, **(_run_kwargs or {}))
    out = np.concatenate([res.results[i]["out"] for i in core_ids], axis=0)
    if _run_kwargs:
        kernel._last_results = res
    return out



# revision 6
# speedup vs baseline: 2.0437x; 2.0437x over previous
"""Trainium2 Bass kernel for nn_PizzaBurningEffect.

Reference computation (per batch b):
    ew[h,w]   : fixed edge-weight grid (input-independent)
    spots     = max_s exp(-((x_w-sx)^2+(y_h-sy)^2)/(2 r_s^2)) * sint_s
    bm        = clip(max(ew, spots) * burn_b, 0, 1)
    out[c]    = img[c] * (1 - kappa_c * burn_b * max(ew, spots)),
                kappa_c = 1 - dark_c
(The clips are no-ops: every operand is in [0,1) and bm <= 0.8.)

Device strategy (p-norm max on the tensor engine):
    max_s g_s ~= (sum_s g_s^32)^(1/32)
The 32nd powers are separable: g_s^32 = gyp_s(h) * gxp_s(w), with the tiny
1-D tables gyp/gxp computed on the host (scaled by sqrt(LAM)=3.16e18 each so
fp32/bf16 dynamic range covers g in [0.017, 1]; smaller factors flush to 0).
Per 128-row chunk the sum over s is ONE 8x128 x 8x512 bf16 matmul into PSUM.
The 1/32 root is a single ACT Exp on the *bitcast-int32* view of the PSUM
sum: the int32 pattern of an fp32 is linear in log2 (max bit-log error
0.086 log2 / 32 -> <0.1% after centring), so Exp(scale*I + bias) with
scale = ln2/(32*2^23) computes (S/LAM)^(1/32) over the full fp32 range.
(ACT's Ln table clamps below 3e-20 and is garbage above 2.5e19, so a real
Ln+Exp root cannot cover the 70-decade range of the 32nd powers.)  A small
deflation delta folded into the Exp bias centres the p-norm overshoot.
DVE then does: max with ew (fp16), three tensor_scalar ops for
F_c = 1 - kappa_c*burn*bm (4x packed), and one wide 16-bit multiply.

img/out travel as bf16 (fp16's subnormal range breaks the rel-err metric at
tiny image values), host-packed into chunk-contiguous [b,k,p,c*w] layout so
every img/out DMA is one 128x1.5KB contiguous block.

Sharding: pure data parallel, 4 batches per core on 8 cores.
"""

import numpy as np
import ml_dtypes

import concourse.bacc as bacc
import concourse.bass as bass
from concourse import mybir
from concourse.tile import TileContext
from concourse.bass_utils import run_bass_kernel_spmd

B, C, H, W, S = 32, 3, 512, 512, 8
NCORES = 8
BL = B // NCORES          # batches per core
P = 128                   # partitions
K = H // P                # row chunks per image
SR = S                   # matmul contraction rows
DT = mybir.dt.float32
DTH = mybir.dt.float16    # mask chain
DTB = mybir.dt.bfloat16   # img/out + power tables
NPB = ml_dtypes.bfloat16

BURN_MIN, BURN_MAX = 0.2, 0.8
DARK = np.array([0.7, 0.4, 0.3], dtype=np.float64)

PNORM = 32.0
LAM = 1e37                # sum scale; sqrt(LAM) per 1-D factor
DELTA = 0.0065            # deflation centring the p-norm overshoot
SIGMA = -0.0430           # bit-log centring constant
EXP_SCALE = float(np.log(2.0) / (PNORM * 2.0 ** 23))
EXP_BIAS = float(np.log(2.0) * (-127.0 - SIGMA) / PNORM
                 - np.log(LAM) / PNORM + np.log1p(-DELTA))


def _build_program():
    nc = bacc.Bacc("TRN2", target_bir_lowering=False, debug=False,
                   num_devices=NCORES)

    img = nc.dram_tensor("img", [BL, K, P, C * W], DTB, kind="ExternalInput")
    gxp = nc.dram_tensor("gxp", [SR, BL, W], DTB, kind="ExternalInput")
    gyp = nc.dram_tensor("gyp", [SR, BL, K, P], DTB, kind="ExternalInput")
    ew = nc.dram_tensor("ew", [P, K, W], DTH, kind="ExternalInput")
    s1 = nc.dram_tensor("s1", [P, BL, C], DT, kind="ExternalInput")
    out = nc.dram_tensor("out", [BL, K, P, C * W], DTB, kind="ExternalOutput")

    mx = mybir.AluOpType.max
    mult = mybir.AluOpType.mult
    add = mybir.AluOpType.add

    with TileContext(nc) as tc:
        with (
            tc.tile_pool(name="singles", bufs=1) as singles,
            tc.tile_pool(name="imgp", bufs=4) as imgp,
            tc.tile_pool(name="outp", bufs=4) as outp,
            tc.tile_pool(name="spp", bufs=4) as spp,
            tc.tile_pool(name="bmp", bufs=4) as bmp,
            tc.tile_pool(name="fp", bufs=4) as fpool,
            tc.psum_pool(name="qp", bufs=4) as qpool,
        ):
            gyp_t = singles.tile([SR, BL, K, P], DTB)
            nc.sync.dma_start(out=gyp_t[:], in_=gyp[:])
            gxp_t = singles.tile([SR, BL, W], DTB)
            nc.sync.dma_start(out=gxp_t[:], in_=gxp[:])
            s1_t = singles.tile([P, BL, C], DT)
            nc.sync.dma_start(out=s1_t[:], in_=s1[:])
            ew_t = singles.tile([P, K, W], DTH)
            nc.sync.dma_start(out=ew_t[:, 0, :], in_=ew[:, 0, :])
            for k in range(1, K):
                nc.sync.dma_start(out=ew_t[:, k, :], in_=ew[:, k, :])

            bias_t = singles.tile([P, 1], DT)
            nc.vector.memset(bias_t[:], EXP_BIAS)

            # Warm the natural_log_exp ACT table set during the initial DMAs.
            warm = singles.tile([P, 2], DT)
            nc.vector.memset(warm[:, 0:1], 1.0)
            nc.scalar.activation(out=warm[:, 1:2], in_=warm[:, 0:1],
                                 func=mybir.ActivationFunctionType.Exp)

            for b in range(BL):
                for k in range(K):
                    img_t = imgp.tile([P, C * W], DTB)
                    nc.sync.dma_start(out=img_t[:], in_=img[b, k])

                    # q = sum_s gyp_s(p) * gxp_s(w)   (PE outer-product sum)
                    q = qpool.tile([P, W], DT)
                    nc.tensor.matmul(q[:], gyp_t[:, b, k, :], gxp_t[:, b, :])

                    # spots = (q/LAM)^(1/32) * (1-DELTA): one Exp on the
                    # bitcast-int32 PSUM view (bit-trick logarithm)
                    sp = spp.tile([P, W], DTH)
                    nc.scalar.activation(
                        out=sp[:], in_=q[:].bitcast(mybir.dt.int32),
                        func=mybir.ActivationFunctionType.Exp,
                        bias=bias_t[:], scale=EXP_SCALE)

                    # bm = max(spots, ew)
                    bm = bmp.tile([P, W], DTH)
                    nc.vector.tensor_tensor(
                        out=bm[:], in0=sp[:], in1=ew_t[:, k, :], op=mx)

                    # F_c = 1 - kappa_c*burn_b*bm: two channels on DVE
                    # (4x-packed tensor_scalar), one on ACT to balance load
                    f_t = fpool.tile([P, C, W], DTH)
                    for c in range(2):
                        nc.vector.tensor_scalar(
                            out=f_t[:, c, :], in0=bm[:],
                            scalar1=s1_t[:, b, c:c + 1], scalar2=1.0,
                            op0=mult, op1=add)
                    nc.scalar.activation(
                        out=f_t[:, 2, :], in_=bm[:],
                        func=mybir.ActivationFunctionType.Identity,
                        bias=1.0, scale=s1_t[:, b, 2:3])

                    # out = img * F   (one wide 16-bit multiply)
                    out_t = outp.tile([P, C * W], DTB)
                    nc.vector.tensor_tensor(
                        out=out_t[:], in0=img_t[:],
                        in1=f_t[:].rearrange("p c w -> p (c w)"), op=mult)

                    nc.sync.dma_start(out=out[b, k], in_=out_t[:])

    nc.compile()
    return nc


_NC = None


def _get_nc():
    global _NC
    if _NC is None:
        _NC = _build_program()
    return _NC


def _host_tables(u_xy, u_radius, u_intensity, u_burn):
    """1-D 32nd-power tables (float64 host math, bf16 on device)."""
    u_xy = np.asarray(u_xy, np.float64)
    u_radius = np.asarray(u_radius, np.float64)
    u_intensity = np.asarray(u_intensity, np.float64)
    u_burn = np.asarray(u_burn, np.float64)

    y = np.linspace(-1.0, 1.0, H)
    x = np.linspace(-1.0, 1.0, W)

    spot_xy = 2.0 * u_xy - 1.0
    sx = spot_xy[..., 0]                   # [B,S]
    sy = spot_xy[..., 1]
    radius = 0.05 + 0.15 * u_radius
    sint = 0.5 + 0.5 * u_intensity
    inv2r2 = 1.0 / (2.0 * radius ** 2)
    burn = BURN_MIN + (BURN_MAX - BURN_MIN) * u_burn   # [B]

    lamh_log = 0.5 * np.log(LAM)
    # log of (sint*gx)^32 * sqrt(LAM) and gy^32 * sqrt(LAM)
    tx = PNORM * (-((x[None, None, :] - sx[..., None]) ** 2)
                  * inv2r2[..., None] + np.log(sint)[..., None]) + lamh_log
    ty = PNORM * (-((y[None, None, :] - sy[..., None]) ** 2)
                  * inv2r2[..., None]) + lamh_log
    gxp = np.where(tx > -87.0, np.exp(tx), 0.0)        # [B,S,W]
    gyp = np.where(ty > -87.0, np.exp(ty), 0.0)        # [B,S,H]

    # device layouts
    gxp_lay = np.ascontiguousarray(
        gxp.transpose(1, 0, 2)).astype(NPB)            # [SR,B,W]
    gyp_lay = np.ascontiguousarray(
        gyp.reshape(B, SR, K, P).transpose(1, 0, 2, 3)).astype(NPB)

    kappa = 1.0 - DARK                                 # [C]
    s1 = -(burn[:, None] * kappa[None, :])             # [B,C]
    s1_lay = np.ascontiguousarray(np.broadcast_to(
        s1.astype(np.float32), (P, B, C)))
    return gxp_lay, gyp_lay, s1_lay


def _edge_weight():
    y = np.linspace(-1.0, 1.0, H)
    x = np.linspace(-1.0, 1.0, W)
    yc, xc = np.meshgrid(y, x, indexing="ij")
    dist = np.sqrt(xc ** 2 + yc ** 2)
    ew = np.exp(2.0 * (dist - 0.7))
    ew = (ew - ew.min()) / (ew.max() - ew.min() + 1e-6)
    # ew_lay[p, k, w] = ew[k*P+p, w]
    return np.ascontiguousarray(
        ew.reshape(K, P, W).transpose(1, 0, 2).astype(np.float16))


_EW = None


def kernel(img, u_xy, u_radius, u_intensity, u_burn, _run_kwargs=None):
    global _EW
    img = np.asarray(img, np.float32)
    # pack to [B, K, P, C*W] bf16: chunk-contiguous DMA blocks
    img_dev = np.ascontiguousarray(
        img.reshape(B, C, K, P, W).transpose(0, 2, 3, 1, 4)
    ).astype(NPB).reshape(B, K, P, C * W)

    gxp_lay, gyp_lay, s1_lay = _host_tables(
        u_xy, u_radius, u_intensity, u_burn)
    if _EW is None:
        _EW = _edge_weight()

    nc = _get_nc()
    core_ids = list(range(NCORES))
    in_maps = []
    for i in core_ids:
        lo, hi = i * BL, (i + 1) * BL
        in_maps.append({
            "img": img_dev[lo:hi],
            "gxp": np.ascontiguousarray(gxp_lay[:, lo:hi]),
            "gyp": np.ascontiguousarray(gyp_lay[:, lo:hi]),
            "ew": _EW,
            "s1": np.ascontiguousarray(s1_lay[:, lo:hi]),
        })
    res = run_bass_kernel_spmd(nc, in_maps, core_ids, **(_run_kwargs or {}))
    out_dev = np.concatenate(
        [np.asarray(res.results[i]["out"]) for i in core_ids], axis=0)
    out = np.ascontiguousarray(
        out_dev.reshape(B, K, P, C, W).transpose(0, 3, 1, 2, 4)
    ).astype(np.float32).reshape(B, C, H, W)
    if _run_kwargs:
        kernel._last_results = res
    return out


# revision 7
# speedup vs baseline: 2.2267x; 1.0895x over previous
"""Trainium2 Bass kernel for nn_PizzaBurningEffect.

Reference computation (per batch b):
    ew[h,w]   : fixed edge-weight grid (input-independent)
    spots     = max_s exp(-((x_w-sx)^2+(y_h-sy)^2)/(2 r_s^2)) * sint_s
    bm        = clip(max(ew, spots) * burn_b, 0, 1)
    out[c]    = img[c] * (1 - kappa_c * burn_b * max(ew, spots)),
                kappa_c = 1 - dark_c
(The clips are no-ops: every operand is in [0,1) and bm <= 0.8.)

Device strategy (p-norm max on the tensor engine):
    max_s g_s ~= (sum_s g_s^32)^(1/32)
The 32nd powers are separable: g_s^32 = gyp_s(h) * gxp_s(w), with the tiny
1-D tables gyp/gxp computed on the host (scaled by sqrt(LAM)=3.16e18 each so
fp32/bf16 dynamic range covers g in [0.017, 1]; smaller factors flush to 0).
Per 128-row chunk the sum over s is ONE 8x128 x 8x512 bf16 matmul into PSUM.
The 1/32 root is a single ACT Exp on the *bitcast-int32* view of the PSUM
sum: the int32 pattern of an fp32 is linear in log2 (max bit-log error
0.086 log2 / 32 -> <0.1% after centring), so Exp(scale*I + bias) with
scale = ln2/(32*2^23) computes (S/LAM)^(1/32) over the full fp32 range.
(ACT's Ln table clamps below 3e-20 and is garbage above 2.5e19, so a real
Ln+Exp root cannot cover the 70-decade range of the 32nd powers.)  A small
deflation delta folded into the Exp bias centres the p-norm overshoot.
DVE then does: max with ew (fp16), three tensor_scalar ops for
F_c = 1 - kappa_c*burn*bm (4x packed), and one wide 16-bit multiply.

img/out travel as bf16 (fp16's subnormal range breaks the rel-err metric at
tiny image values), host-packed into chunk-contiguous [b,k,p,c*w] layout so
every img/out DMA is large contiguous 3KB-per-partition blocks.  TRN2 has
two HW-DGE rings (one fed by the SP engine, one by ACT): img loads go on
the SP ring, table/ew loads and output stores on the ACT ring, so load and
store traffic move in parallel.

Sharding: pure data parallel, 4 batches per core on 8 cores.
"""

import numpy as np
import ml_dtypes

import concourse.bacc as bacc
import concourse.bass as bass
from concourse import mybir
from concourse.tile import TileContext
from concourse.bass_utils import run_bass_kernel_spmd

B, C, H, W, S = 32, 3, 512, 512, 8
NCORES = 8
BL = B // NCORES          # batches per core
P = 128                   # partitions
K = H // P                # row chunks per image
SR = S                   # matmul contraction rows
DT = mybir.dt.float32
DTH = mybir.dt.float16    # mask chain
DTB = mybir.dt.bfloat16   # img/out + power tables
NPB = ml_dtypes.bfloat16

BURN_MIN, BURN_MAX = 0.2, 0.8
DARK = np.array([0.7, 0.4, 0.3], dtype=np.float64)

PNORM = 32.0
LAM = 1e37                # sum scale; sqrt(LAM) per 1-D factor
DELTA = 0.0065            # deflation centring the p-norm overshoot
SIGMA = -0.0430           # bit-log centring constant
EXP_SCALE = float(np.log(2.0) / (PNORM * 2.0 ** 23))
EXP_BIAS = float(np.log(2.0) * (-127.0 - SIGMA) / PNORM
                 - np.log(LAM) / PNORM + np.log1p(-DELTA))


def _build_program():
    nc = bacc.Bacc("TRN2", target_bir_lowering=False, debug=False,
                   num_devices=NCORES)

    img = nc.dram_tensor("img", [BL, K, P, C * W], DTB, kind="ExternalInput")
    gxp = nc.dram_tensor("gxp", [SR, BL, W], DTB, kind="ExternalInput")
    gyp = nc.dram_tensor("gyp", [SR, BL, K, P], DTB, kind="ExternalInput")
    ew = nc.dram_tensor("ew", [P, K, W], DTH, kind="ExternalInput")
    s1 = nc.dram_tensor("s1", [P, BL, C], DT, kind="ExternalInput")
    out = nc.dram_tensor("out", [BL, K, P, C * W], DTB, kind="ExternalOutput")

    mx = mybir.AluOpType.max
    mult = mybir.AluOpType.mult
    add = mybir.AluOpType.add

    with TileContext(nc) as tc:
        with (
            tc.tile_pool(name="singles", bufs=1) as singles,
            tc.tile_pool(name="imgp", bufs=4) as imgp,
            tc.tile_pool(name="outp", bufs=4) as outp,
            tc.tile_pool(name="spp", bufs=4) as spp,
            tc.tile_pool(name="bmp", bufs=4) as bmp,
            tc.tile_pool(name="fp", bufs=4) as fpool,
            tc.psum_pool(name="qp", bufs=4) as qpool,
        ):
            gyp_t = singles.tile([SR, BL, K, P], DTB)
            nc.scalar.dma_start(out=gyp_t[:], in_=gyp[:])
            gxp_t = singles.tile([SR, BL, W], DTB)
            nc.scalar.dma_start(out=gxp_t[:], in_=gxp[:])
            s1_t = singles.tile([P, BL, C], DT)
            nc.scalar.dma_start(out=s1_t[:], in_=s1[:])
            ew_t = singles.tile([P, K, W], DTH)
            nc.scalar.dma_start(out=ew_t[:, 0, :], in_=ew[:, 0, :])
            for k in range(1, K):
                nc.scalar.dma_start(out=ew_t[:, k, :], in_=ew[:, k, :])

            bias_t = singles.tile([P, 1], DT)
            nc.vector.memset(bias_t[:], EXP_BIAS)

            # Warm the natural_log_exp ACT table set during the initial DMAs.
            warm = singles.tile([P, 2], DT)
            nc.vector.memset(warm[:, 0:1], 1.0)
            nc.scalar.activation(out=warm[:, 1:2], in_=warm[:, 0:1],
                                 func=mybir.ActivationFunctionType.Exp)

            for b in range(BL):
                for j in range(K // 2):
                    # two row-chunks per DMA (fewer, larger transfers)
                    img_t = imgp.tile([P, 2, C * W], DTB)
                    nc.sync.dma_start(
                        out=img_t[:],
                        in_=img[b, 2 * j:2 * j + 2].rearrange(
                            "k p f -> p k f"))
                    out_t = outp.tile([P, 2, C * W], DTB)

                    for kk in range(2):
                        k = 2 * j + kk
                        # q = sum_s gyp_s(p) * gxp_s(w)  (PE outer products)
                        q = qpool.tile([P, W], DT)
                        nc.tensor.matmul(
                            q[:], gyp_t[:, b, k, :], gxp_t[:, b, :])

                        # spots = (q/LAM)^(1/32) * (1-DELTA): one Exp on the
                        # bitcast-int32 PSUM view (bit-trick logarithm)
                        sp = spp.tile([P, W], DTH)
                        nc.scalar.activation(
                            out=sp[:], in_=q[:].bitcast(mybir.dt.int32),
                            func=mybir.ActivationFunctionType.Exp,
                            bias=bias_t[:], scale=EXP_SCALE)

                        # bm = max(spots, ew)
                        bm = bmp.tile([P, W], DTH)
                        nc.vector.tensor_tensor(
                            out=bm[:], in0=sp[:], in1=ew_t[:, k, :], op=mx)

                        # F_c = 1 - kappa_c*burn_b*bm: two channels on DVE
                        # (4x-packed tensor_scalar), one on ACT
                        f_t = fpool.tile([P, C, W], DTH)
                        for c in range(2):
                            nc.vector.tensor_scalar(
                                out=f_t[:, c, :], in0=bm[:],
                                scalar1=s1_t[:, b, c:c + 1], scalar2=1.0,
                                op0=mult, op1=add)
                        nc.scalar.activation(
                            out=f_t[:, 2, :], in_=bm[:],
                            func=mybir.ActivationFunctionType.Identity,
                            bias=1.0, scale=s1_t[:, b, 2:3])

                        # out = img * F   (one wide 16-bit multiply)
                        nc.vector.tensor_tensor(
                            out=out_t[:, kk, :], in0=img_t[:, kk, :],
                            in1=f_t[:].rearrange("p c w -> p (c w)"),
                            op=mult)

                    nc.scalar.dma_start(
                        out=out[b, 2 * j:2 * j + 2].rearrange(
                            "k p f -> p k f"),
                        in_=out_t[:])

    nc.compile()
    return nc


_NC = None


def _get_nc():
    global _NC
    if _NC is None:
        _NC = _build_program()
    return _NC


def _host_tables(u_xy, u_radius, u_intensity, u_burn):
    """1-D 32nd-power tables (float64 host math, bf16 on device)."""
    u_xy = np.asarray(u_xy, np.float64)
    u_radius = np.asarray(u_radius, np.float64)
    u_intensity = np.asarray(u_intensity, np.float64)
    u_burn = np.asarray(u_burn, np.float64)

    y = np.linspace(-1.0, 1.0, H)
    x = np.linspace(-1.0, 1.0, W)

    spot_xy = 2.0 * u_xy - 1.0
    sx = spot_xy[..., 0]                   # [B,S]
    sy = spot_xy[..., 1]
    radius = 0.05 + 0.15 * u_radius
    sint = 0.5 + 0.5 * u_intensity
    inv2r2 = 1.0 / (2.0 * radius ** 2)
    burn = BURN_MIN + (BURN_MAX - BURN_MIN) * u_burn   # [B]

    lamh_log = 0.5 * np.log(LAM)
    # log of (sint*gx)^32 * sqrt(LAM) and gy^32 * sqrt(LAM)
    tx = PNORM * (-((x[None, None, :] - sx[..., None]) ** 2)
                  * inv2r2[..., None] + np.log(sint)[..., None]) + lamh_log
    ty = PNORM * (-((y[None, None, :] - sy[..., None]) ** 2)
                  * inv2r2[..., None]) + lamh_log
    gxp = np.where(tx > -87.0, np.exp(tx), 0.0)        # [B,S,W]
    gyp = np.where(ty > -87.0, np.exp(ty), 0.0)        # [B,S,H]

    # device layouts
    gxp_lay = np.ascontiguousarray(
        gxp.transpose(1, 0, 2)).astype(NPB)            # [SR,B,W]
    gyp_lay = np.ascontiguousarray(
        gyp.reshape(B, SR, K, P).transpose(1, 0, 2, 3)).astype(NPB)

    kappa = 1.0 - DARK                                 # [C]
    s1 = -(burn[:, None] * kappa[None, :])             # [B,C]
    s1_lay = np.ascontiguousarray(np.broadcast_to(
        s1.astype(np.float32), (P, B, C)))
    return gxp_lay, gyp_lay, s1_lay


def _edge_weight():
    y = np.linspace(-1.0, 1.0, H)
    x = np.linspace(-1.0, 1.0, W)
    yc, xc = np.meshgrid(y, x, indexing="ij")
    dist = np.sqrt(xc ** 2 + yc ** 2)
    ew = np.exp(2.0 * (dist - 0.7))
    ew = (ew - ew.min()) / (ew.max() - ew.min() + 1e-6)
    # ew_lay[p, k, w] = ew[k*P+p, w]
    return np.ascontiguousarray(
        ew.reshape(K, P, W).transpose(1, 0, 2).astype(np.float16))


_EW = None


def kernel(img, u_xy, u_radius, u_intensity, u_burn, _run_kwargs=None):
    global _EW
    img = np.asarray(img, np.float32)
    # pack to [B, K, P, C*W] bf16: chunk-contiguous DMA blocks
    img_dev = np.ascontiguousarray(
        img.reshape(B, C, K, P, W).transpose(0, 2, 3, 1, 4)
    ).astype(NPB).reshape(B, K, P, C * W)

    gxp_lay, gyp_lay, s1_lay = _host_tables(
        u_xy, u_radius, u_intensity, u_burn)
    if _EW is None:
        _EW = _edge_weight()

    nc = _get_nc()
    core_ids = list(range(NCORES))
    in_maps = []
    for i in core_ids:
        lo, hi = i * BL, (i + 1) * BL
        in_maps.append({
            "img": img_dev[lo:hi],
            "gxp": np.ascontiguousarray(gxp_lay[:, lo:hi]),
            "gyp": np.ascontiguousarray(gyp_lay[:, lo:hi]),
            "ew": _EW,
            "s1": np.ascontiguousarray(s1_lay[:, lo:hi]),
        })
    res = run_bass_kernel_spmd(nc, in_maps, core_ids, **(_run_kwargs or {}))
    out_dev = np.concatenate(
        [np.asarray(res.results[i]["out"]) for i in core_ids], axis=0)
    out = np.ascontiguousarray(
        out_dev.reshape(B, K, P, C, W).transpose(0, 3, 1, 2, 4)
    ).astype(np.float32).reshape(B, C, H, W)
    if _run_kwargs:
        kernel._last_results = res
    return out


# revision 8
# speedup vs baseline: 2.3771x; 1.0676x over previous
"""Trainium2 Bass kernel for nn_PizzaBurningEffect.

Reference computation (per batch b):
    ew[h,w]   : fixed edge-weight grid (input-independent)
    spots     = max_s exp(-((x_w-sx)^2+(y_h-sy)^2)/(2 r_s^2)) * sint_s
    bm        = clip(max(ew, spots) * burn_b, 0, 1)
    out[c]    = img[c] * (1 - kappa_c * burn_b * max(ew, spots)),
                kappa_c = 1 - dark_c
(The clips are no-ops: every operand is in [0,1) and bm <= 0.8.)

Device strategy (p-norm max on the tensor engine):
    max_s g_s ~= (sum_s g_s^32)^(1/32)
The 32nd powers are separable: g_s^32 = gyp_s(h) * gxp_s(w), with the tiny
1-D tables gyp/gxp computed on the host (scaled by sqrt(LAM)=3.16e18 each so
fp32/bf16 dynamic range covers g in [0.017, 1]; smaller factors flush to 0).
Per 128-row chunk the sum over s is ONE 8x128 x 8x512 bf16 matmul into PSUM.
The 1/32 root is a single ACT Exp on the *bitcast-int32* view of the PSUM
sum: the int32 pattern of an fp32 is linear in log2 (max bit-log error
0.086 log2 / 32 -> <0.1% after centring), so Exp(scale*I + bias) with
scale = ln2/(32*2^23) computes (S/LAM)^(1/32) over the full fp32 range.
(ACT's Ln table clamps below 3e-20 and is garbage above 2.5e19, so a real
Ln+Exp root cannot cover the 70-decade range of the 32nd powers.)  A small
deflation delta folded into the Exp bias centres the p-norm overshoot.
DVE then does: max with ew (fp16), three tensor_scalar ops for
F_c = 1 - kappa_c*burn*bm (4x packed), and one wide 16-bit multiply.

img/out travel as bf16 (fp16's subnormal range breaks the rel-err metric at
tiny image values), host-packed into chunk-contiguous [b,k,p,c*w] layout so
every img/out DMA is large contiguous 3KB-per-partition blocks.  TRN2 has
two HW-DGE rings (one fed by the SP engine, one by ACT): img loads go on
the SP ring, table/ew loads and output stores on the ACT ring, so load and
store traffic move in parallel.

Sharding: pure data parallel, 4 batches per core on 8 cores.
"""

import numpy as np
import ml_dtypes

import concourse.bacc as bacc
import concourse.bass as bass
from concourse import mybir
from concourse.tile import TileContext
from concourse.bass_utils import run_bass_kernel_spmd

B, C, H, W, S = 32, 3, 512, 512, 8
NCORES = 8
BL = B // NCORES          # batches per core
P = 128                   # partitions
K = H // P                # row chunks per image
SR = S                   # matmul contraction rows
DT = mybir.dt.float32
DTH = mybir.dt.float16    # mask chain
DTB = mybir.dt.bfloat16   # img/out + power tables
NPB = ml_dtypes.bfloat16

BURN_MIN, BURN_MAX = 0.2, 0.8
DARK = np.array([0.7, 0.4, 0.3], dtype=np.float64)

PNORM = 32.0
LAM = 1e37                # sum scale; sqrt(LAM) per 1-D factor
DELTA = 0.0065            # deflation centring the p-norm overshoot
SIGMA = -0.0430           # bit-log centring constant
EXP_SCALE = float(np.log(2.0) / (PNORM * 2.0 ** 23))
EXP_BIAS = float(np.log(2.0) * (-127.0 - SIGMA) / PNORM
                 - np.log(LAM) / PNORM + np.log1p(-DELTA))


def _build_program():
    nc = bacc.Bacc("TRN2", target_bir_lowering=False, debug=False,
                   num_devices=NCORES)

    img = nc.dram_tensor("img", [BL, K, P, C * W], DTB, kind="ExternalInput")
    gxp = nc.dram_tensor("gxp", [SR, BL, W], DTB, kind="ExternalInput")
    gyp = nc.dram_tensor("gyp", [SR, BL, K, P], DTB, kind="ExternalInput")
    ew = nc.dram_tensor("ew", [P, K, W], DTH, kind="ExternalInput")
    s1 = nc.dram_tensor("s1", [P, BL, C], DT, kind="ExternalInput")
    out = nc.dram_tensor("out", [BL, K, P, C * W], DTB, kind="ExternalOutput")

    mx = mybir.AluOpType.max
    mult = mybir.AluOpType.mult
    add = mybir.AluOpType.add

    with TileContext(nc) as tc:
        with (
            tc.tile_pool(name="singles", bufs=1) as singles,
            tc.tile_pool(name="imgp", bufs=4) as imgp,
            tc.tile_pool(name="outp", bufs=4) as outp,
            tc.tile_pool(name="spp", bufs=4) as spp,
            tc.tile_pool(name="bmp", bufs=4) as bmp,
            tc.tile_pool(name="fp", bufs=4) as fpool,
            tc.psum_pool(name="qp", bufs=3) as qpool,
        ):
            gyp_t = singles.tile([SR, BL, K, P], DTB)
            nc.scalar.dma_start(out=gyp_t[:], in_=gyp[:])
            gxp_t = singles.tile([SR, BL, W], DTB)
            nc.scalar.dma_start(out=gxp_t[:], in_=gxp[:])
            s1_t = singles.tile([P, BL, C], DT)
            nc.scalar.dma_start(out=s1_t[:], in_=s1[:])
            ew_t = singles.tile([P, K, W], DTH)
            nc.scalar.dma_start(out=ew_t[:, 0, :], in_=ew[:, 0, :])
            for k in range(1, K):
                nc.scalar.dma_start(out=ew_t[:, k, :], in_=ew[:, k, :])

            bias_t = singles.tile([P, 1], DT)
            nc.vector.memset(bias_t[:], EXP_BIAS)

            # Warm the natural_log_exp ACT table set during the initial DMAs.
            warm = singles.tile([P, 2], DT)
            nc.vector.memset(warm[:, 0:1], 1.0)
            nc.scalar.activation(out=warm[:, 1:2], in_=warm[:, 0:1],
                                 func=mybir.ActivationFunctionType.Exp)

            for b in range(BL):
                for j in range(K // 2):
                    # two row-chunks per tile; all compute ops span both
                    # chunks to amortize per-instruction overhead
                    img_t = imgp.tile([P, 2, C * W], DTB)
                    for kk in range(2):
                        nc.sync.dma_start(
                            out=img_t[:, kk, :],
                            in_=img[b, 2 * j + kk])

                    # q = sum_s gyp_s(p) * gxp_s(w)  (PE outer products)
                    q = qpool.tile([P, 2, W], DT)
                    for kk in range(2):
                        nc.tensor.matmul(
                            q[:, kk, :], gyp_t[:, b, 2 * j + kk, :],
                            gxp_t[:, b, :])

                    # spots = (q/LAM)^(1/32) * (1-DELTA): one Exp on the
                    # bitcast-int32 PSUM view (bit-trick logarithm)
                    sp = spp.tile([P, 2, W], DTH)
                    nc.scalar.activation(
                        out=sp[:], in_=q[:].bitcast(mybir.dt.int32),
                        func=mybir.ActivationFunctionType.Exp,
                        bias=bias_t[:], scale=EXP_SCALE)

                    # bm = max(spots, ew)
                    bm = bmp.tile([P, 2, W], DTH)
                    nc.vector.tensor_tensor(
                        out=bm[:], in0=sp[:],
                        in1=ew_t[:, 2 * j:2 * j + 2, :], op=mx)

                    # F_c = 1 - kappa_c*burn_b*bm: two channels on DVE
                    # (4x-packed tensor_scalar), one on ACT to balance load
                    f_t = fpool.tile([P, 2, C, W], DTH)
                    for c in range(2):
                        nc.vector.tensor_scalar(
                            out=f_t[:, :, c, :], in0=bm[:],
                            scalar1=s1_t[:, b, c:c + 1], scalar2=1.0,
                            op0=mult, op1=add)
                    nc.scalar.activation(
                        out=f_t[:, :, 2, :], in_=bm[:],
                        func=mybir.ActivationFunctionType.Identity,
                        bias=1.0, scale=s1_t[:, b, 2:3])

                    # out = img * F   (one wide 16-bit multiply)
                    out_t = outp.tile([P, 2, C * W], DTB)
                    nc.vector.tensor_tensor(
                        out=out_t[:], in0=img_t[:],
                        in1=f_t[:].rearrange("p k c w -> p k (c w)"),
                        op=mult)

                    nc.scalar.dma_start(
                        out=out[b, 2 * j:2 * j + 2].rearrange(
                            "k p f -> p k f"),
                        in_=out_t[:])

    nc.compile()
    return nc


_NC = None


def _get_nc():
    global _NC
    if _NC is None:
        _NC = _build_program()
    return _NC


def _host_tables(u_xy, u_radius, u_intensity, u_burn):
    """1-D 32nd-power tables (float64 host math, bf16 on device)."""
    u_xy = np.asarray(u_xy, np.float64)
    u_radius = np.asarray(u_radius, np.float64)
    u_intensity = np.asarray(u_intensity, np.float64)
    u_burn = np.asarray(u_burn, np.float64)

    y = np.linspace(-1.0, 1.0, H)
    x = np.linspace(-1.0, 1.0, W)

    spot_xy = 2.0 * u_xy - 1.0
    sx = spot_xy[..., 0]                   # [B,S]
    sy = spot_xy[..., 1]
    radius = 0.05 + 0.15 * u_radius
    sint = 0.5 + 0.5 * u_intensity
    inv2r2 = 1.0 / (2.0 * radius ** 2)
    burn = BURN_MIN + (BURN_MAX - BURN_MIN) * u_burn   # [B]

    lamh_log = 0.5 * np.log(LAM)
    # log of (sint*gx)^32 * sqrt(LAM) and gy^32 * sqrt(LAM)
    tx = PNORM * (-((x[None, None, :] - sx[..., None]) ** 2)
                  * inv2r2[..., None] + np.log(sint)[..., None]) + lamh_log
    ty = PNORM * (-((y[None, None, :] - sy[..., None]) ** 2)
                  * inv2r2[..., None]) + lamh_log
    gxp = np.where(tx > -87.0, np.exp(tx), 0.0)        # [B,S,W]
    gyp = np.where(ty > -87.0, np.exp(ty), 0.0)        # [B,S,H]

    # device layouts
    gxp_lay = np.ascontiguousarray(
        gxp.transpose(1, 0, 2)).astype(NPB)            # [SR,B,W]
    gyp_lay = np.ascontiguousarray(
        gyp.reshape(B, SR, K, P).transpose(1, 0, 2, 3)).astype(NPB)

    kappa = 1.0 - DARK                                 # [C]
    s1 = -(burn[:, None] * kappa[None, :])             # [B,C]
    s1_lay = np.ascontiguousarray(np.broadcast_to(
        s1.astype(np.float32), (P, B, C)))
    return gxp_lay, gyp_lay, s1_lay


def _edge_weight():
    y = np.linspace(-1.0, 1.0, H)
    x = np.linspace(-1.0, 1.0, W)
    yc, xc = np.meshgrid(y, x, indexing="ij")
    dist = np.sqrt(xc ** 2 + yc ** 2)
    ew = np.exp(2.0 * (dist - 0.7))
    ew = (ew - ew.min()) / (ew.max() - ew.min() + 1e-6)
    # ew_lay[p, k, w] = ew[k*P+p, w]
    return np.ascontiguousarray(
        ew.reshape(K, P, W).transpose(1, 0, 2).astype(np.float16))


_EW = None


def kernel(img, u_xy, u_radius, u_intensity, u_burn, _run_kwargs=None):
    global _EW
    img = np.asarray(img, np.float32)
    # pack to [B, K, P, C*W] bf16: chunk-contiguous DMA blocks
    img_dev = np.ascontiguousarray(
        img.reshape(B, C, K, P, W).transpose(0, 2, 3, 1, 4)
    ).astype(NPB).reshape(B, K, P, C * W)

    gxp_lay, gyp_lay, s1_lay = _host_tables(
        u_xy, u_radius, u_intensity, u_burn)
    if _EW is None:
        _EW = _edge_weight()

    nc = _get_nc()
    core_ids = list(range(NCORES))
    in_maps = []
    for i in core_ids:
        lo, hi = i * BL, (i + 1) * BL
        in_maps.append({
            "img": img_dev[lo:hi],
            "gxp": np.ascontiguousarray(gxp_lay[:, lo:hi]),
            "gyp": np.ascontiguousarray(gyp_lay[:, lo:hi]),
            "ew": _EW,
            "s1": np.ascontiguousarray(s1_lay[:, lo:hi]),
        })
    res = run_bass_kernel_spmd(nc, in_maps, core_ids, **(_run_kwargs or {}))
    out_dev = np.concatenate(
        [np.asarray(res.results[i]["out"]) for i in core_ids], axis=0)
    out = np.ascontiguousarray(
        out_dev.reshape(B, K, P, C, W).transpose(0, 3, 1, 2, 4)
    ).astype(np.float32).reshape(B, C, H, W)
    if _run_kwargs:
        kernel._last_results = res
    return out
